# revision 30
# baseline (speedup 1.0000x reference)
"""Grayscale + single-level 2x2 Haar DWT kernel for Trainium2 (8 cores, SPMD).

Full input x [16,3,1024,1024] f32 -> out [16,4,512,512] f32.
Batch-sharded: core i handles samples [2i, 2i+1].

Math per sample (BGR weights w=(0.114,0.587,0.299), all bands scaled by 0.5):
  gray = w0*x[0] + w1*x[1] + w2*x[2]
  a,b,c,d = gray[0::2,0::2], gray[0::2,1::2], gray[1::2,0::2], gray[1::2,1::2]
  cA,cH,cV,cD = 0.5*(a+b+c+d), 0.5*(a+b-c-d), 0.5*(a-b+c-d), 0.5*(a-b-c+d)

Structure: uniform pieces of 256 consecutive input rows, loaded as one
contiguous 1MB DMA per channel into [128, 2, 1024] tiles (partition p
holds input rows 2p, 2p+1 of the piece -> output row p). 4 pieces per
sample plane, 8 per core. Outputs store as contiguous 256KB per band.

Engine split (per piece):
  ACT : c_i = x_i * (w_i/2) cast f32->fp16, channels 0/1 (ACTIVATE-Copy
        with folded scale)
  DVE : t = c0+c1 (fp16 dense TT, 2x_1p mode)
        g = ch2_f32*(w2/2) + t (scalar_tensor_tensor, 1x - folds the
        third channel's scale+cast, keeping ACT off the drain path)
        srow/drow = g[:,0,:] +/- g[:,1,:] (fp16 dense 2x, into t's rows)
        cA,cV = srow[...,0::2] +/- [...,1::2] (f32 out, 1x)
  GPSIMD: cH,cD = drow[...,0::2] +/- [...,1::2], plus all store
        descriptor gen (SWDGE ring - never blocks loads or compute)
  SYNC: load descriptor gen only (nothing ever blocks the load ring)

Pipelined with 4-deep input buffering; DMA is the bottleneck
(~33.5 MB/core at ~430 GB/s sustained, the SBUF-fabric ceiling).
"""

import numpy as np

N_CORES = 8
B, C, H, W = 16, 3, 1024, 1024
HO, WO = H // 2, W // 2
SPC = B // N_CORES   # samples per core
PCS = H // 256       # pieces per sample plane (256 input rows each)

W_BGR = (0.114, 0.587, 0.299)

_compiled = None


def _build():
    from concourse import bacc, mybir
    from concourse.tile import TileContext

    f32 = mybir.dt.float32
    f16 = mybir.dt.float16
    add = mybir.AluOpType.add
    sub = mybir.AluOpType.subtract
    mult = mybir.AluOpType.mult

    w0h, w1h, w2h = (w * 0.5 for w in W_BGR)

    nc = bacc.Bacc("TRN2", target_bir_lowering=False, debug=False)
    # same bytes as [SPC, C, H, W] f32, pre-shaped for piece DMA
    x = nc.declare_dram_parameter(
        "x", [SPC, C, PCS, 128, 2, W], f32, isOutput=False
    )
    out = nc.declare_dram_parameter(
        "out", [SPC, 4, PCS, 128, 1, WO], f32, isOutput=True
    )

    pieces = [(s, p) for s in range(SPC) for p in range(PCS)]
    n = len(pieces)

    with TileContext(nc) as tc:
        with (
            tc.tile_pool(name="in_pool", bufs=4) as in_pool,
            tc.tile_pool(name="sc_pool", bufs=4) as sc_pool,
            tc.tile_pool(name="mid_pool", bufs=4) as mid_pool,
            tc.tile_pool(name="out_pool", bufs=3) as out_pool,
        ):
            in_tiles = {}

            def issue_loads(i):
                s, p = pieces[i]
                ts = []
                for ch in range(C):
                    t = in_pool.tile(
                        [128, 2, W], f32, tag=f"in{ch}", name=f"in{ch}_{i}"
                    )
                    nc.sync.dma_start(out=t[:, :, :], in_=x[s, ch, p])
                    ts.append(t)
                in_tiles[i] = ts

            def compute_and_store(i):
                s, p = pieces[i]
                ch_t = in_tiles.pop(i)
                sc0 = sc_pool.tile([128, 2, W], f16, tag="sc0", name=f"sc0_{i}")
                sc1 = sc_pool.tile([128, 2, W], f16, tag="sc1", name=f"sc1_{i}")
                t = mid_pool.tile([128, 2, W], f16, tag="t", name=f"t_{i}")
                nc.scalar.mul(sc0[:, :, :], ch_t[0][:, :, :], w0h)
                nc.scalar.mul(sc1[:, :, :], ch_t[1][:, :, :], w1h)
                nc.vector.tensor_tensor(
                    t[:, :, :], sc0[:, :, :], sc1[:, :, :], add
                )
                g = sc0  # dead after the TT; reuse its slot
                if i % 2 == 0:
                    # STT folds ch2's scale+cast on DVE (1x)
                    nc.vector.scalar_tensor_tensor(
                        g[:, :, :], ch_t[2][:, :, :], w2h, t[:, :, :], mult, add
                    )
                else:
                    # ACT-heavy form: third ACTIVATE + cheap 2x fp16 TT.
                    # Alternating forms balances DVE vs ACT so neither
                    # engine falls behind the DMA stream when throttled.
                    sc2 = sc_pool.tile(
                        [128, 2, W], f16, tag="sc2", name=f"sc2_{i}", bufs=3
                    )
                    nc.scalar.mul(sc2[:, :, :], ch_t[2][:, :, :], w2h)
                    nc.vector.tensor_tensor(
                        g[:, :, :], t[:, :, :], sc2[:, :, :], add
                    )
                # stage 1: row combine into t's rows (t is dead after g)
                srow, drow = t[:, 0:1, :], t[:, 1:2, :]
                nc.vector.tensor_tensor(srow, g[:, 0:1, :], g[:, 1:2, :], add)
                nc.vector.tensor_tensor(drow, g[:, 0:1, :], g[:, 1:2, :], sub)
                # stage 2: column combine (strided, f32 out) + store.
                # cA/cV on DVE, cH/cD on gpsimd - cuts the DVE critical
                # backlog; gpsimd interleaves them with its store desc-gen.
                for band, (src, op, eng) in enumerate(
                    (
                        (srow, add, nc.vector),
                        (drow, add, nc.gpsimd),
                        (srow, sub, nc.vector),
                        (drow, sub, nc.gpsimd),
                    )
                ):
                    # band order: cA, cH, cV, cD
                    o = out_pool.tile(
                        [128, 1, WO], f32, tag=f"o{band}", name=f"o{band}_{i}"
                    )
                    eng.tensor_tensor(
                        o[:, :, :], src[:, :, 0:W:2], src[:, :, 1:W:2], op
                    )
                    if i == n - 1:
                        # final piece: desc-gen on the idle ACT HWDGE ring,
                        # overlapping gpsimd's stage-2 ops in the drain
                        nc.scalar.dma_start(out=out[s, band, p], in_=o[:, :, :])
                    else:
                        # SWDGE (gpsimd Q7) ring: store desc-gen never blocks
                        # the load ring or the compute engines
                        nc.gpsimd.dma_start(out=out[s, band, p], in_=o[:, :, :])

            for i in range(n):
                issue_loads(i)
                if i >= 2:
                    compute_and_store(i - 2)
            compute_and_store(n - 2)
            compute_and_store(n - 1)
    nc.finalize()
    return nc


def kernel(x: np.ndarray) -> np.ndarray:
    global _compiled
    from concourse.bass_utils import run_bass_kernel_spmd

    if _compiled is None:
        _compiled = _build()
    nc = _compiled

    x = np.ascontiguousarray(x, dtype=np.float32)
    in_maps = [{"x": x[i * SPC : (i + 1) * SPC]} for i in range(N_CORES)]
    res = run_bass_kernel_spmd(nc, in_maps, list(range(N_CORES))).results
    out = np.concatenate(
        [r["out"].reshape(SPC, 4, HO, WO) for r in res], axis=0
    )
    return out


# revision 32
# speedup vs baseline: 1.0284x; 1.0284x over previous
"""Grayscale + single-level 2x2 Haar DWT kernel for Trainium2 (8 cores, SPMD).

Full input x [16,3,1024,1024] f32 -> out [16,4,512,512] f32.
Batch-sharded: core i handles samples [2i, 2i+1].

Math per sample (BGR weights w=(0.114,0.587,0.299), all bands scaled by 0.5):
  gray = w0*x[0] + w1*x[1] + w2*x[2]
  a,b,c,d = gray[0::2,0::2], gray[0::2,1::2], gray[1::2,0::2], gray[1::2,1::2]
  cA,cH,cV,cD = 0.5*(a+b+c+d), 0.5*(a+b-c-d), 0.5*(a-b+c-d), 0.5*(a-b-c+d)

Structure: uniform pieces of 256 consecutive input rows, loaded as one
contiguous 1MB DMA per channel into [128, 2, 1024] tiles (partition p
holds input rows 2p, 2p+1 of the piece -> output row p). 4 pieces per
sample plane, 8 per core. Outputs store as contiguous 256KB per band.

Engine split (per piece):
  ACT : c_i = x_i * (w_i/2) cast f32->fp16, channels 0/1 (ACTIVATE-Copy
        with folded scale)
  DVE : t = c0+c1 (fp16 dense TT, 2x_1p mode)
        g = ch2_f32*(w2/2) + t (scalar_tensor_tensor, 1x - folds the
        third channel's scale+cast, keeping ACT off the drain path)
        srow/drow = g[:,0,:] +/- g[:,1,:] (fp16 dense 2x, into t's rows)
        cA,cV = srow[...,0::2] +/- [...,1::2] (f32 out, 1x)
  GPSIMD: cH,cD = drow[...,0::2] +/- [...,1::2], plus all store
        descriptor gen (SWDGE ring - never blocks loads or compute)
  SYNC: load descriptor gen only (nothing ever blocks the load ring)

Pipelined with 4-deep input buffering; DMA is the bottleneck
(~33.5 MB/core at ~430 GB/s sustained, the SBUF-fabric ceiling).
"""

import numpy as np

N_CORES = 8
B, C, H, W = 16, 3, 1024, 1024
HO, WO = H // 2, W // 2
SPC = B // N_CORES   # samples per core
PCS = H // 256       # pieces per sample plane (256 input rows each)

W_BGR = (0.114, 0.587, 0.299)

_compiled = None


def _build():
    from concourse import bacc, mybir
    from concourse.tile import TileContext

    f32 = mybir.dt.float32
    f16 = mybir.dt.float16
    add = mybir.AluOpType.add
    sub = mybir.AluOpType.subtract
    mult = mybir.AluOpType.mult

    w0h, w1h, w2h = (w * 0.5 for w in W_BGR)

    nc = bacc.Bacc("TRN2", target_bir_lowering=False, debug=False)
    # same bytes as [SPC, C, H, W] f32, pre-shaped for piece DMA
    x = nc.declare_dram_parameter(
        "x", [SPC, C, PCS, 128, 2, W], f32, isOutput=False
    )
    out = nc.declare_dram_parameter(
        "out", [SPC, 4, PCS, 128, 1, WO], f32, isOutput=True
    )

    pieces = [(s, p) for s in range(SPC) for p in range(PCS)]
    n = len(pieces)

    with TileContext(nc) as tc:
        with (
            tc.tile_pool(name="in_pool", bufs=4) as in_pool,
            tc.tile_pool(name="sc_pool", bufs=4) as sc_pool,
            tc.tile_pool(name="mid_pool", bufs=4) as mid_pool,
            tc.tile_pool(name="out_pool", bufs=3) as out_pool,
        ):
            in_tiles = {}

            def issue_loads(i):
                s, p = pieces[i]
                ts = []
                for ch in range(C):
                    t = in_pool.tile(
                        [128, 2, W], f32, tag=f"in{ch}", name=f"in{ch}_{i}"
                    )
                    nc.sync.dma_start(out=t[:, :, :], in_=x[s, ch, p])
                    ts.append(t)
                in_tiles[i] = ts

            def compute_and_store(i):
                s, p = pieces[i]
                ch_t = in_tiles.pop(i)
                sc0 = sc_pool.tile([128, 2, W], f16, tag="sc0", name=f"sc0_{i}")
                sc1 = sc_pool.tile([128, 2, W], f16, tag="sc1", name=f"sc1_{i}")
                t = mid_pool.tile([128, 2, W], f16, tag="t", name=f"t_{i}", bufs=3)
                nc.scalar.mul(sc0[:, :, :], ch_t[0][:, :, :], w0h)
                nc.scalar.mul(sc1[:, :, :], ch_t[1][:, :, :], w1h)
                nc.vector.tensor_tensor(
                    t[:, :, :], sc0[:, :, :], sc1[:, :, :], add
                )
                g = sc0  # dead after the TT; reuse its slot
                if i % 2 == 0:
                    # STT folds ch2's scale+cast on DVE (1x)
                    nc.vector.scalar_tensor_tensor(
                        g[:, :, :], ch_t[2][:, :, :], w2h, t[:, :, :], mult, add
                    )
                else:
                    # ACT-heavy form: third ACTIVATE + cheap 2x fp16 TT.
                    # Alternating forms balances DVE vs ACT so neither
                    # engine falls behind the DMA stream when throttled.
                    sc2 = sc_pool.tile(
                        [128, 2, W], f16, tag="sc2", name=f"sc2_{i}", bufs=3
                    )
                    nc.scalar.mul(sc2[:, :, :], ch_t[2][:, :, :], w2h)
                    nc.vector.tensor_tensor(
                        g[:, :, :], t[:, :, :], sc2[:, :, :], add
                    )
                # stage 1: row combine. srow/drow get their own DEEP rings:
                # their last readers include gpsimd's stage-2 ops, and a
                # shallow ring here would couple the DVE front-end to
                # gpsimd's (late-running) pipeline position.
                srow = mid_pool.tile(
                    [128, 1, W], f16, tag="srow", name=f"srow_{i}", bufs=6
                )
                drow = mid_pool.tile(
                    [128, 1, W], f16, tag="drow", name=f"drow_{i}", bufs=6
                )
                srow, drow = srow[:, :, :], drow[:, :, :]
                nc.vector.tensor_tensor(srow, g[:, 0:1, :], g[:, 1:2, :], add)
                nc.vector.tensor_tensor(drow, g[:, 0:1, :], g[:, 1:2, :], sub)
                # stage 2: column combine (strided, f32 out) + store.
                # cA/cV on DVE, cH/cD on gpsimd - cuts the DVE critical
                # backlog; gpsimd interleaves them with its store desc-gen.
                for band, (src, op, eng) in enumerate(
                    (
                        (srow, add, nc.vector),
                        (drow, add, nc.gpsimd),
                        (srow, sub, nc.vector),
                        (drow, sub, nc.gpsimd),
                    )
                ):
                    # band order: cA, cH, cV, cD
                    o = out_pool.tile(
                        [128, 1, WO], f32, tag=f"o{band}", name=f"o{band}_{i}"
                    )
                    eng.tensor_tensor(
                        o[:, :, :], src[:, :, 0:W:2], src[:, :, 1:W:2], op
                    )
                    if i == n - 1:
                        # final piece: desc-gen on the idle ACT HWDGE ring,
                        # overlapping gpsimd's stage-2 ops in the drain
                        nc.scalar.dma_start(out=out[s, band, p], in_=o[:, :, :])
                    else:
                        # SWDGE (gpsimd Q7) ring: store desc-gen never blocks
                        # the load ring or the compute engines
                        nc.gpsimd.dma_start(out=out[s, band, p], in_=o[:, :, :])

            for i in range(n):
                issue_loads(i)
                if i >= 2:
                    compute_and_store(i - 2)
            compute_and_store(n - 2)
            compute_and_store(n - 1)
    nc.finalize()
    return nc


def kernel(x: np.ndarray) -> np.ndarray:
    global _compiled
    from concourse.bass_utils import run_bass_kernel_spmd

    if _compiled is None:
        _compiled = _build()
    nc = _compiled

    x = np.ascontiguousarray(x, dtype=np.float32)
    in_maps = [{"x": x[i * SPC : (i + 1) * SPC]} for i in range(N_CORES)]
    res = run_bass_kernel_spmd(nc, in_maps, list(range(N_CORES))).results
    out = np.concatenate(
        [r["out"].reshape(SPC, 4, HO, WO) for r in res], axis=0
    )
    return out


# revision 33
# speedup vs baseline: 1.1762x; 1.1437x over previous
"""Grayscale + single-level 2x2 Haar DWT kernel for Trainium2 (8 cores, SPMD).

Full input x [16,3,1024,1024] f32 -> out [16,4,512,512] f32.
Batch-sharded: core i handles samples [2i, 2i+1].

Math per sample (BGR weights w=(0.114,0.587,0.299), all bands scaled by 0.5):
  gray = w0*x[0] + w1*x[1] + w2*x[2]
  a,b,c,d = gray[0::2,0::2], gray[0::2,1::2], gray[1::2,0::2], gray[1::2,1::2]
  cA,cH,cV,cD = 0.5*(a+b+c+d), 0.5*(a+b-c-d), 0.5*(a-b+c-d), 0.5*(a-b-c+d)

Structure: uniform pieces of 256 consecutive input rows, loaded as one
contiguous 1MB DMA per channel into [128, 2, 1024] tiles (partition p
holds input rows 2p, 2p+1 of the piece -> output row p). 4 pieces per
sample plane, 8 per core. Outputs store as contiguous 256KB per band.

Engine split (per piece):
  ACT : c_i = x_i * (w_i/2) cast f32->fp16, channels 0/1 (ACTIVATE-Copy
        with folded scale)
  DVE : t = c0+c1 (fp16 dense TT, 2x_1p mode)
        g = ch2_f32*(w2/2) + t (scalar_tensor_tensor, 1x - folds the
        third channel's scale+cast, keeping ACT off the drain path)
        srow/drow = g[:,0,:] +/- g[:,1,:] (fp16 dense 2x, into t's rows)
        cA,cV = srow[...,0::2] +/- [...,1::2] (f32 out, 1x)
  GPSIMD: cH,cD = drow[...,0::2] +/- [...,1::2], plus all store
        descriptor gen (SWDGE ring - never blocks loads or compute)
  SYNC: load descriptor gen only (nothing ever blocks the load ring)

Pipelined with 4-deep input buffering; DMA is the bottleneck
(~33.5 MB/core at ~430 GB/s sustained, the SBUF-fabric ceiling).
"""

import numpy as np

N_CORES = 8
B, C, H, W = 16, 3, 1024, 1024
HO, WO = H // 2, W // 2
SPC = B // N_CORES   # samples per core
PCS = H // 256       # pieces per sample plane (256 input rows each)

W_BGR = (0.114, 0.587, 0.299)

_compiled = None


def _build():
    from concourse import bacc, mybir
    from concourse.tile import TileContext

    f32 = mybir.dt.float32
    f16 = mybir.dt.float16
    add = mybir.AluOpType.add
    sub = mybir.AluOpType.subtract
    mult = mybir.AluOpType.mult

    w0h, w1h, w2h = (w * 0.5 for w in W_BGR)

    nc = bacc.Bacc("TRN2", target_bir_lowering=False, debug=False)
    # same bytes as [SPC, C, H, W] f32, pre-shaped for piece DMA
    x = nc.declare_dram_parameter(
        "x", [SPC, C, PCS, 128, 2, W], f32, isOutput=False
    )
    out = nc.declare_dram_parameter(
        "out", [SPC, 4, PCS, 128, 1, WO], f32, isOutput=True
    )

    pieces = [(s, p) for s in range(SPC) for p in range(PCS)]
    n = len(pieces)

    with TileContext(nc) as tc:
        with (
            tc.tile_pool(name="in_pool", bufs=4) as in_pool,
            tc.tile_pool(name="sc_pool", bufs=4) as sc_pool,
            tc.tile_pool(name="mid_pool", bufs=4) as mid_pool,
            tc.tile_pool(name="out_pool", bufs=3) as out_pool,
        ):
            in_tiles = {}

            def issue_loads(i):
                s, p = pieces[i]
                ts = []
                for ch in range(C):
                    t = in_pool.tile(
                        [128, 2, W], f32, tag=f"in{ch}", name=f"in{ch}_{i}"
                    )
                    nc.sync.dma_start(out=t[:, :, :], in_=x[s, ch, p])
                    ts.append(t)
                in_tiles[i] = ts

            def compute_and_store(i):
                s, p = pieces[i]
                ch_t = in_tiles.pop(i)
                sc0 = sc_pool.tile([128, 2, W], f16, tag="sc0", name=f"sc0_{i}")
                sc1 = sc_pool.tile([128, 2, W], f16, tag="sc1", name=f"sc1_{i}")
                t = mid_pool.tile([128, 2, W], f16, tag="t", name=f"t_{i}", bufs=3)
                nc.scalar.mul(sc0[:, :, :], ch_t[0][:, :, :], w0h)
                nc.scalar.mul(sc1[:, :, :], ch_t[1][:, :, :], w1h)
                nc.vector.tensor_tensor(
                    t[:, :, :], sc0[:, :, :], sc1[:, :, :], add
                )
                g = sc0  # dead after the TT; reuse its slot
                if i % 2 == 1:
                    # STT folds ch2's scale+cast on DVE (1x)
                    nc.vector.scalar_tensor_tensor(
                        g[:, :, :], ch_t[2][:, :, :], w2h, t[:, :, :], mult, add
                    )
                else:
                    # ACT-heavy form: third ACTIVATE + cheap 2x fp16 TT.
                    # Alternating forms balances DVE vs ACT so neither
                    # engine falls behind the DMA stream when throttled.
                    sc2 = sc_pool.tile(
                        [128, 2, W], f16, tag="sc2", name=f"sc2_{i}", bufs=3
                    )
                    nc.scalar.mul(sc2[:, :, :], ch_t[2][:, :, :], w2h)
                    nc.vector.tensor_tensor(
                        g[:, :, :], t[:, :, :], sc2[:, :, :], add
                    )
                # stage 1: row combine. srow/drow get their own DEEP rings:
                # their last readers include gpsimd's stage-2 ops, and a
                # shallow ring here would couple the DVE front-end to
                # gpsimd's (late-running) pipeline position.
                srow = mid_pool.tile(
                    [128, 1, W], f16, tag="srow", name=f"srow_{i}", bufs=6
                )
                drow = mid_pool.tile(
                    [128, 1, W], f16, tag="drow", name=f"drow_{i}", bufs=6
                )
                srow, drow = srow[:, :, :], drow[:, :, :]
                nc.vector.tensor_tensor(srow, g[:, 0:1, :], g[:, 1:2, :], add)
                nc.vector.tensor_tensor(drow, g[:, 0:1, :], g[:, 1:2, :], sub)
                # stage 2: column combine (strided, f32 out) + store.
                # cA/cV on DVE, cH/cD on gpsimd - cuts the DVE critical
                # backlog; gpsimd interleaves them with its store desc-gen.
                for band, (src, op, eng) in enumerate(
                    (
                        (srow, add, nc.vector),
                        (drow, add, nc.gpsimd),
                        (srow, sub, nc.vector),
                        (drow, sub, nc.gpsimd),
                    )
                ):
                    # band order: cA, cH, cV, cD
                    o = out_pool.tile(
                        [128, 1, WO], f32, tag=f"o{band}", name=f"o{band}_{i}"
                    )
                    eng.tensor_tensor(
                        o[:, :, :], src[:, :, 0:W:2], src[:, :, 1:W:2], op
                    )
                    if i == n - 1:
                        # final piece: desc-gen on the idle ACT HWDGE ring,
                        # overlapping gpsimd's stage-2 ops in the drain
                        nc.scalar.dma_start(out=out[s, band, p], in_=o[:, :, :])
                    else:
                        # SWDGE (gpsimd Q7) ring: store desc-gen never blocks
                        # the load ring or the compute engines
                        nc.gpsimd.dma_start(out=out[s, band, p], in_=o[:, :, :])

            for i in range(n):
                issue_loads(i)
                if i >= 2:
                    compute_and_store(i - 2)
            compute_and_store(n - 2)
            compute_and_store(n - 1)
    nc.finalize()
    return nc


def kernel(x: np.ndarray) -> np.ndarray:
    global _compiled
    from concourse.bass_utils import run_bass_kernel_spmd

    if _compiled is None:
        _compiled = _build()
    nc = _compiled

    x = np.ascontiguousarray(x, dtype=np.float32)
    in_maps = [{"x": x[i * SPC : (i + 1) * SPC]} for i in range(N_CORES)]
    res = run_bass_kernel_spmd(nc, in_maps, list(range(N_CORES))).results
    out = np.concatenate(
        [r["out"].reshape(SPC, 4, HO, WO) for r in res], axis=0
    )
    return out
